# revision 2
# baseline (speedup 1.0000x reference)
"""GAT (3-layer, PyG-style) Trainium2 Bass kernel, sharded across 8 NeuronCores.

v2: dst-node-range sharding (graph parallel), per-layer pipeline:
  - layer 0: every core computes the FULL h0 table locally from the full x
    input (redundant compute beats the AllGather), so layer 0 needs no
    collective at all.
  - layers 1/2: each core computes h_ext for its own nodes; AllGather in
    lo/hi halves (int16 gather-index limit), kicked early by processing
    dst tiles hi-range-first so the collective overlaps aggregation.
  - aggregation per layer runs in two phases by SOURCE half: phase 1
    gathers hi-half sources for all 49 dst tiles and stores numerator /
    denominator partials in SBUF (bf16/f32); phase 2 gathers lo-half,
    combines, softmax-normalizes, applies ELU (bias-folded: the next
    layer consumes ELU+1 and its bias absorbs -colsum(W)), and computes
    the next layer's h_ext tile immediately (dataflow-interleaved).
  - edge softmax: exp(leakyrelu(al_s[src]+al_d[dst])) via the scalar
    engine's Lrelu/Exp; al_d[dst] per edge via S^T-transpose matmuls;
    the weighted scatter-add is selection-matrix matmuls in PSUM.
  - all matmuls bf16 (fp32 accumulation in PSUM).

kernel(**inputs) takes FULL inputs, returns the FULL [N, 16] output.
"""

import sys

sys.path.insert(0, "/opt/trn_rl_repo")

import numpy as np

import concourse.bass as bass
import concourse.mybir as mybir
import concourse.tile as tile
from concourse import bacc
from concourse import bass_utils
from concourse.bass_interp import get_hw_module
from concourse.masks import make_identity
from concourse import library_config

F32 = mybir.dt.float32
BF = mybir.dt.bfloat16
I16 = mybir.dt.int16
import ml_dtypes
NPBF = ml_dtypes.bfloat16
P = 128


def real_cfg():
    R = 8
    N = 50000
    PER = N // R                      # 6250 nodes per core
    T = (PER + P - 1) // P            # 49 dst tiles per core
    return dict(
        R=R, N=N, PER=PER, T=T, NPAD=T * P,
        F_IN=128, HID=64, HEADS=8, N_CLASSES=16,
        NEG=0.2, SPLIT_T=25, B_LO=5, B_HI=5,
    )


# ---------------------------------------------------------------------------
# Host-side preprocessing
# ---------------------------------------------------------------------------

def _wrap16(flat):
    """int16 index list -> dma_gather idx layout [128, n/16]."""
    n = flat.shape[-1]
    w = flat.reshape(flat.shape[:-1] + (n // 16, 16))      # [..., c, 16]
    w = np.swapaxes(w, -1, -2)                             # [..., 16, c]
    reps = (1,) * (flat.ndim - 1) + (8, 1)
    return np.ascontiguousarray(np.tile(w, reps), np.int16)  # [..., 128, c]


def host_prepare(inputs, cfg):
    R, N, PER, T, NPAD = cfg["R"], cfg["N"], cfg["PER"], cfg["T"], cfg["NPAD"]
    F_IN, HID, HEADS, NCLS = cfg["F_IN"], cfg["HID"], cfg["HEADS"], cfg["N_CLASSES"]
    HC = HID * HEADS
    SPLIT_T = cfg["SPLIT_T"]
    LO = SPLIT_T * P
    HI = NPAD - LO
    B_LO, B_HI = cfg["B_LO"], cfg["B_HI"]
    B = B_LO + B_HI

    x = np.asarray(inputs["x"], np.float32)
    ei = np.asarray(inputs["edge_index"])
    src = ei[0].astype(np.int64)
    dst = ei[1].astype(np.int64)   # self-loops handled analytically on device

    core = dst // PER
    dloc = (dst - core * PER).astype(np.int64)
    sloc = (src % PER).astype(np.int64)
    srank = (src // PER).astype(np.int64)
    is_lo = sloc < LO
    tile_of = dloc // P

    cl = np.zeros((R, T), np.int64)
    ch = np.zeros((R, T), np.int64)
    np.add.at(cl, (core[is_lo], tile_of[is_lo]), 1)
    np.add.at(ch, (core[~is_lo], tile_of[~is_lo]), 1)
    assert cl.max() <= B_LO * P and ch.max() <= B_HI * P, (cl.max(), ch.max())

    idx_lo = np.zeros((R, T, B_LO * P), np.int16)
    idx_hi = np.zeros((R, T, B_HI * P), np.int16)
    dlc = np.full((R, T, P, B), -1.0, np.float32)

    grow = np.where(is_lo, srank * LO + sloc, srank * HI + (sloc - LO))

    order = np.lexsort((~is_lo * 1, tile_of, core))
    g_s = grow[order]
    d_s = dloc[order]
    core_s = core[order]
    tile_s = tile_of[order]
    lo_s = is_lo[order]

    grp = core_s * (2 * T) + tile_s * 2 + (~lo_s).astype(np.int64)
    grp_start = np.searchsorted(grp, np.arange(R * T * 2), side="left")
    pos = np.arange(len(grp)) - grp_start[grp]

    lo_m = lo_s
    hi_m = ~lo_s
    idx_lo[core_s[lo_m], tile_s[lo_m], pos[lo_m]] = g_s[lo_m].astype(np.int16)
    idx_hi[core_s[hi_m], tile_s[hi_m], pos[hi_m]] = g_s[hi_m].astype(np.int16)
    fpos = np.where(lo_m, pos, B_LO * P + pos)
    dlc[core_s, tile_s, fpos % P, fpos // P] = (d_s - tile_s * P).astype(np.float32)
    dlc = dlc.astype(NPBF)

    idx_lo = _wrap16(idx_lo)     # [R, T, 128, B_LO*8]
    idx_hi = _wrap16(idx_hi)
    # replicated-row layout for St builds: [R, T, 128, B*128]
    dlcr = np.ascontiguousarray(np.broadcast_to(
        dlc.astype(np.float32).transpose(0, 1, 3, 2).reshape(R, T, 1, B * P),
        (R, T, P, B * P))).astype(NPBF)

    ROWG = 640
    ROWG2 = 128

    def wext(W, a_s, a_d, ncols):
        Fin = W.shape[0]
        H, C = a_s.shape
        Wr = W.reshape(Fin, H, C)
        We = np.zeros((Fin, ncols), np.float32)
        We[:, : H * C] = W
        We[:, H * C : H * C + H] = np.einsum("fhc,hc->fh", Wr, a_s)
        We[:, H * C + H : H * C + 2 * H] = np.einsum("fhc,hc->fh", Wr, a_d)
        return We

    W0e = wext(np.asarray(inputs["W0"], np.float32),
               np.asarray(inputs["a_s0"], np.float32),
               np.asarray(inputs["a_d0"], np.float32), ROWG)
    W1e = wext(np.asarray(inputs["W1"], np.float32),
               np.asarray(inputs["a_s1"], np.float32),
               np.asarray(inputs["a_d1"], np.float32), ROWG)
    W2e = wext(np.asarray(inputs["W2"], np.float32),
               np.asarray(inputs["a_s2"], np.float32),
               np.asarray(inputs["a_d2"], np.float32), ROWG2)

    # ELU bias fold: layers 1/2 consume E' = elu(x)+1, so subtract colsum(We).
    def bext(b, We, ncols, fold):
        be = np.zeros((1, ncols), np.float32)
        be[0, : b.shape[0]] = b
        if fold:
            be[0, :] -= We.sum(axis=0)
        return np.ascontiguousarray(np.broadcast_to(be, (P, ncols)))

    # tables are stored WITHOUT bias; bias (incl. ELU fold on h cols) is
    # added post-normalize, and the attention-column folds ride in alc.
    b0e = bext(np.asarray(inputs["b0"], np.float32), W0e, ROWG, False)
    b1e = bext(np.asarray(inputs["b1"], np.float32), W1e, ROWG, True)
    b2e = bext(np.asarray(inputs["b2"], np.float32), W2e, ROWG2, True)
    # al-fold vectors: layer L's ald slot gets fold_s + fold_d of that layer
    def afold(We, ncols, nH):
        f = np.zeros((1, 16), np.f4 if False else np.float32)
        cs = -We.sum(axis=0)
        f[0, nH:2 * nH] = cs[ncols:ncols + nH] + cs[ncols + nH:ncols + 2 * nH]
        return np.ascontiguousarray(np.broadcast_to(f, (P, 16)))
    af1 = afold(W1e, 512, 8)
    af2 = afold(W2e, 16, 1)

    W0e = W0e.astype(NPBF)
    W1e_r = W1e.reshape(4, P, ROWG).transpose(1, 0, 2).astype(NPBF).copy()
    W2e_r = W2e.reshape(4, P, ROWG2).transpose(1, 0, 2).astype(NPBF).copy()

    # full x, feature-major, in table order [128, R, NPAD] (zero pad cols)
    xtf = np.zeros((F_IN, R, NPAD), np.float32)
    xtf[:, :, :PER] = x.T.reshape(F_IN, R, PER)
    xtf = xtf.astype(NPBF)

    in_maps = []
    for r in range(R):
        xt0 = np.ascontiguousarray(xtf[:, r, :])
        in_maps.append({
            "xtf": xtf, "xt0": xt0,
            "w0e": W0e, "w1e": W1e_r, "w2e": W2e_r,
            "b0e": b0e, "b1e": b1e, "b2e": b2e,
            "af1": af1, "af2": af2,
            "idx_lo": idx_lo[r], "idx_hi": idx_hi[r],
            "dlc": dlc[r], "dlcr": dlcr[r],
        })
    return in_maps


# ---------------------------------------------------------------------------
# Device program
# ---------------------------------------------------------------------------

def build_gat_nc(cfg):
    R, PER, T, NPAD = cfg["R"], cfg["PER"], cfg["T"], cfg["NPAD"]
    F_IN, HID, HEADS, NCLS = cfg["F_IN"], cfg["HID"], cfg["HEADS"], cfg["N_CLASSES"]
    NEG = cfg["NEG"]
    B_LO, B_HI = cfg["B_LO"], cfg["B_HI"]
    B = B_LO + B_HI
    ROWG = 640
    ROWG2 = 128
    SPLIT_T = cfg["SPLIT_T"]
    LO = SPLIT_T * P
    HI = NPAD - LO

    nc = bacc.Bacc("TRN2", target_bir_lowering=False, debug=False,
                   num_devices=R)

    xtf_d = nc.dram_tensor("xtf", [F_IN, R, NPAD], BF, kind="ExternalInput")
    xt0_d = nc.dram_tensor("xt0", [F_IN, NPAD], BF, kind="ExternalInput")
    w0e_d = nc.dram_tensor("w0e", [F_IN, ROWG], BF, kind="ExternalInput")
    w1e_d = nc.dram_tensor("w1e", [P, 4, ROWG], BF, kind="ExternalInput")
    w2e_d = nc.dram_tensor("w2e", [P, 4, ROWG2], BF, kind="ExternalInput")
    b0e_d = nc.dram_tensor("b0e", [P, ROWG], F32, kind="ExternalInput")
    b1e_d = nc.dram_tensor("b1e", [P, ROWG], F32, kind="ExternalInput")
    b2e_d = nc.dram_tensor("b2e", [P, ROWG2], F32, kind="ExternalInput")
    ilo_d = nc.dram_tensor("idx_lo", [T, P, B_LO * 8], I16, kind="ExternalInput")
    ihi_d = nc.dram_tensor("idx_hi", [T, P, B_HI * 8], I16, kind="ExternalInput")
    dlc_d = nc.dram_tensor("dlc", [T, P, B], BF, kind="ExternalInput")
    dlcr_d = nc.dram_tensor("dlcr", [T, P, B * P], BF, kind="ExternalInput")
    af1_d = nc.dram_tensor("af1", [P, 16], F32, kind="ExternalInput")
    af2_d = nc.dram_tensor("af2", [P, 16], F32, kind="ExternalInput")
    out_d = nc.dram_tensor("out", [PER, NCLS], F32, kind="ExternalOutput")

    rg = [list(range(R))]
    LRELU = mybir.ActivationFunctionType.Lrelu
    EXP = mybir.ActivationFunctionType.Exp
    RELU = mybir.ActivationFunctionType.Relu
    ADD = mybir.AluOpType.add
    MUL = mybir.AluOpType.mult
    ISEQ = mybir.AluOpType.is_equal

    with tile.TileContext(nc) as tc:
        with (
            tc.tile_pool(name="pers", bufs=1) as pers,
            tc.tile_pool(name="sbx", bufs=2) as sbx,
            tc.tile_pool(name="sb", bufs=2) as sb,
            tc.tile_pool(name="sbg", bufs=3) as sbg,
            tc.tile_pool(name="sbh", bufs=4) as sbh,
            tc.tile_pool(name="sbs", bufs=3) as sbs,
            tc.tile_pool(name="ps_h", bufs=2, space="PSUM") as ps_h,
            tc.tile_pool(name="ps_po", bufs=2, space="PSUM") as ps_po,
            tc.tile_pool(name="ps_sm", bufs=1, space="PSUM") as ps_sm,
            tc.tile_pool(name="ps_pt", bufs=1, space="PSUM") as ps_pt,
            tc.tile_pool(name="dram", bufs=1, space="DRAM") as dram,
        ):
            nc.gpsimd.load_library(library_config.mlp)

            # ---- persistent tiles ----
            iota_i = pers.tile([P, P], I16)
            iota_row = pers.tile([P, P], BF)
            iota_col = pers.tile([P, P], BF)
            identb = pers.tile([P, P], BF)
            nc.gpsimd.iota(iota_i[:], pattern=[[1, P]], base=0, channel_multiplier=0)
            nc.vector.tensor_copy(iota_row[:], iota_i[:])
            nc.gpsimd.iota(iota_i[:], pattern=[[0, P]], base=0, channel_multiplier=1)
            nc.vector.tensor_copy(iota_col[:], iota_i[:])
            make_identity(nc, identb[:])
            af1_sb = pers.tile([P, 16], F32)
            af2_sb = pers.tile([P, 16], F32)
            nc.sync.dma_start(af1_sb[:], af1_d[:, :])
            nc.sync.dma_start(af2_sb[:], af2_d[:, :])

            w0_sb = pers.tile([P, ROWG], BF)
            w1_sb = pers.tile([P, 4, ROWG], BF)
            w2_sb = pers.tile([P, 4, ROWG2], BF)
            b0_sb = pers.tile([P, ROWG], F32)
            b1_sb = pers.tile([P, ROWG], F32)
            b2_sb = pers.tile([P, ROWG2], F32)
            nc.sync.dma_start(w0_sb[:], w0e_d[:, :])
            nc.sync.dma_start(w1_sb[:], w1e_d[:, :, :])
            nc.sync.dma_start(w2_sb[:], w2e_d[:, :, :])
            nc.sync.dma_start(b0_sb[:], b0e_d[:, :])
            nc.sync.dma_start(b1_sb[:], b1e_d[:, :])
            nc.sync.dma_start(b2_sb[:], b2e_d[:, :])

            # own als/ald cache per layer (ping-pong): [P, T*16]
            alc = [pers.tile([P, T * 16], BF, name=f"alc{i}") for i in range(2)]
            # hi-phase partials
            part_po = pers.tile([P, T, 512], BF)
            part_den = pers.tile([P, T * 8], F32)

            # ---- internal DRAM ----
            tab0_lo = dram.tile([R * LO, ROWG], BF, name="tab0lo")
            tab0_hi = dram.tile([R * HI, ROWG], BF, name="tab0hi")
            hown0 = dram.tile([NPAD, ROWG], BF, name="hown0")
            hlo = [None,
                   dram.tile([LO, ROWG], BF, name="hlo1"),
                   dram.tile([LO, ROWG2], BF, name="hlo2")]
            hhi = [None,
                   dram.tile([HI, ROWG], BF, name="hhi1"),
                   dram.tile([HI, ROWG2], BF, name="hhi2")]
            tlo = [tab0_lo,
                   dram.tile([R * LO, ROWG], BF, addr_space="Shared", name="tlo1"),
                   dram.tile([R * LO, ROWG2], BF, addr_space="Shared", name="tlo2")]
            thi = [tab0_hi,
                   dram.tile([R * HI, ROWG], BF, addr_space="Shared", name="thi1"),
                   dram.tile([R * HI, ROWG2], BF, addr_space="Shared", name="thi2")]

            # =========== layer-0 table build (local, no collective) ==========
            # own pass first: fills alc[0] and hown0
            xr0 = pers.tile([P, NPAD], BF)
            nc.sync.dma_start(xr0[:], xt0_d[:, :])
            for t in range(T):
                ph = ps_h.tile([P, ROWG], F32, tag="ph")
                nc.tensor.matmul(ph[:, 0:512], lhsT=xr0[:, t * P:(t + 1) * P],
                                 rhs=w0_sb[:, 0:512], start=True, stop=True)
                nc.tensor.matmul(ph[:, 512:528], lhsT=xr0[:, t * P:(t + 1) * P],
                                 rhs=w0_sb[:, 512:528], start=True, stop=True)
                hsb = sbh.tile([P, ROWG], BF, tag="hsb")
                nc.scalar.copy(hsb[:, 0:528], ph[:, 0:528])
                nc.vector.tensor_copy(alc[0][:, t * 16:t * 16 + 16],
                                      hsb[:, 512:528])
                nc.sync.dma_start(hown0[t * P:(t + 1) * P, :], hsb[:])

            # full table: hi half first (phase 1 gathers need it), then lo
            for half in ("hi", "lo"):
                t0, t1 = (SPLIT_T, T) if half == "hi" else (0, SPLIT_T)
                for rr in range(R):
                    xr = sbx.tile([P, NPAD], BF, tag="xr")
                    nc.sync.dma_start(xr[:], xtf_d[:, rr, :])
                    for t in range(t0, t1):
                        ph = ps_h.tile([P, ROWG], F32, tag="ph")
                        nc.tensor.matmul(ph[:, 0:512],
                                         lhsT=xr[:, t * P:(t + 1) * P],
                                         rhs=w0_sb[:, 0:512],
                                         start=True, stop=True)
                        nc.tensor.matmul(ph[:, 512:528],
                                         lhsT=xr[:, t * P:(t + 1) * P],
                                         rhs=w0_sb[:, 512:528],
                                         start=True, stop=True)
                        hsb = sbh.tile([P, ROWG], BF, tag="hsb")
                        nc.scalar.copy(hsb[:, 0:528], ph[:, 0:528])
                        if half == "hi":
                            r0 = rr * HI + t * P - LO
                            nc.sync.dma_start(tab0_hi[r0:r0 + P, :], hsb[:])
                        else:
                            r0 = rr * LO + t * P
                            nc.sync.dma_start(tab0_lo[r0:r0 + P, :], hsb[:])

            # =================== per-layer aggregation ======================
            for L in range(3):
                rowg = ROWG if L < 2 else ROWG2
                nH = HEADS if L < 2 else 1
                ncols = HID * HEADS if L < 2 else NCLS
                alow = ncols            # col of al_s in ext rows
                adoff = ncols + nH      # col of al_d
                a_cur = alc[L % 2]
                a_nxt = alc[(L + 1) % 2]
                rowg_n = ROWG if L == 0 else ROWG2   # next layer's rowg
                W_n = w1_sb if L == 0 else w2_sb
                b_n = b1_sb if L == 0 else b2_sb

                def do_group(t, grp):
                    """Gather + weight one source-group of tile t.
                    Returns (po_psum, pd_psum, S_all, w)."""
                    nb = B_HI if grp == "hi" else B_LO
                    boff = B_LO if grp == "hi" else 0
                    idx_t = ihi_d if grp == "hi" else ilo_d
                    tab = thi[L] if grp == "hi" else tlo[L]

                    idx = sb.tile([P, nb * 8], I16, tag="idx")
                    dlct = sb.tile([P, B], BF, tag="dlct")
                    nc.sync.dma_start(idx[:], idx_t[t, :, :])
                    nc.sync.dma_start(dlct[:], dlc_d[t, :, :])

                    g = sbg.tile([P, nb, rowg], BF, tag="g")
                    nc.gpsimd.dma_gather(
                        g[:], tab[:, :], idx[:],
                        num_idxs=nb * P, num_idxs_reg=nb * P,
                        elem_size=rowg)

                    dlrep = sb.tile([P, nb, P], BF, tag="dlrep")
                    nc.sync.dma_start(
                        dlrep[:], dlcr_d[t, :, boff * P:(boff + nb) * P]
                        .rearrange("p (b q) -> p b q", b=nb))
                    S_all = sbs.tile([P, nb, P], BF, tag="S")
                    nc.vector.tensor_tensor(
                        S_all[:],
                        dlct[:, boff:boff + nb].unsqueeze(2)
                            .to_broadcast([P, nb, P]),
                        iota_row[:].unsqueeze(1).to_broadcast([P, nb, P]),
                        ISEQ)
                    St_all = sbs.tile([P, nb, P], BF, tag="St")
                    nc.vector.tensor_tensor(
                        St_all[:],
                        iota_col[:].unsqueeze(1).to_broadcast([P, nb, P]),
                        dlrep[:],
                        ISEQ)

                    psm = ps_sm.tile([P, nb * nH + nH], F32, tag="psm")
                    pad_ps = psm[:, 0:nb * nH]
                    pd = psm[:, nb * nH:nb * nH + nH]
                    for b in range(nb):
                        nc.tensor.matmul(
                            pad_ps[:, b * nH:(b + 1) * nH],
                            lhsT=St_all[:, b, :],
                            rhs=a_cur[:, t * 16 + nH:t * 16 + 2 * nH],
                            start=True, stop=True)

                    logits = sb.tile([P, nb * nH], F32, tag="logits")
                    nc.vector.tensor_tensor(
                        logits[:].rearrange("p (b h) -> p b h", b=nb),
                        g[:, :, alow:alow + nH],
                        pad_ps[:].rearrange("p (b h) -> p b h", b=nb),
                        ADD)
                    lr = sb.tile([P, nb * nH], F32, tag="lr")
                    import os as _oL
                    if _oL.environ.get("LRELU", "dve") == "act":
                        nc.scalar.activation(lr[:], logits[:], LRELU, alpha=NEG)
                    else:
                        nc.vector.tensor_scalar_mul(lr[:], logits[:], NEG)
                        nc.vector.tensor_tensor(lr[:], lr[:], logits[:],
                                                mybir.AluOpType.max)
                    w = sb.tile([P, nb * nH], BF, tag="w")
                    nc.scalar.activation(w[:], lr[:], EXP)

                    gw = sb.tile([P, nb, ncols], BF, tag="gw")
                    nc.vector.tensor_tensor(
                        gw[:],
                        g[:, :, 0:ncols].rearrange("p b (h c) -> p b h c", h=nH),
                        w[:].rearrange("p (b h) -> p b h", b=nb)
                            .unsqueeze(3)
                            .to_broadcast([P, nb, nH, ncols // nH]),
                        MUL)

                    po = ps_po.tile([P, 512], F32, tag="po")
                    for b in range(nb):
                        nc.tensor.matmul(po[:, 0:ncols], lhsT=S_all[:, b, :],
                                         rhs=gw[:, b, :],
                                         start=(b == 0), stop=(b == nb - 1))
                        nc.tensor.matmul(pd[:], lhsT=S_all[:, b, :],
                                         rhs=w[:, b * nH:(b + 1) * nH],
                                         start=(b == 0), stop=(b == nb - 1))
                    return po, pd

                # ---------- phase 1: hi-half sources -> partials ----------
                for t in range(T):
                    po, pd = do_group(t, "hi")
                    nc.scalar.copy(part_po[:, t, 0:ncols], po[:, 0:ncols])
                    nc.scalar.copy(part_den[:, t * 8:t * 8 + nH], pd[:])

                # ---------- phase 2: lo-half + combine + finalize ----------
                order = list(range(SPLIT_T, T)) + list(range(SPLIT_T))
                for it, t in enumerate(order):
                    po, pd = do_group(t, "lo")

                    # self-loop: ws = exp(lrelu(als_own + ald_own))
                    sl = sb.tile([P, 2 * nH], F32, tag="sl")
                    nc.vector.tensor_tensor(
                        sl[:, 0:nH], a_cur[:, t * 16:t * 16 + nH],
                        a_cur[:, t * 16 + nH:t * 16 + 2 * nH], ADD)
                    import os as _oL2
                    if _oL2.environ.get("LRELU", "dve") == "act":
                        nc.scalar.activation(sl[:, nH:2 * nH], sl[:, 0:nH],
                                             LRELU, alpha=NEG)
                    else:
                        nc.vector.tensor_scalar_mul(sl[:, nH:2 * nH],
                                                    sl[:, 0:nH], NEG)
                        nc.vector.tensor_tensor(sl[:, nH:2 * nH],
                                                sl[:, nH:2 * nH],
                                                sl[:, 0:nH],
                                                mybir.AluOpType.max)
                    ws = sb.tile([P, nH], F32, tag="ws")
                    nc.scalar.activation(ws[:], sl[:, nH:2 * nH], EXP)

                    den = sb.tile([P, 2 * nH], F32, tag="den")
                    nc.vector.tensor_tensor(den[:, 0:nH], pd[:],
                                            part_den[:, t * 8:t * 8 + nH], ADD)
                    nc.vector.tensor_tensor(den[:, 0:nH], den[:, 0:nH], ws[:], ADD)
                    rden = sb.tile([P, nH], F32, tag="rden")
                    nc.vector.reciprocal(rden[:], den[:, 0:nH])
                    wr = sb.tile([P, nH], F32, tag="wr")
                    nc.vector.tensor_tensor(wr[:], ws[:], rden[:], MUL)

                    # own row for the self-loop numerator
                    loc = sb.tile([P, ncols], BF, tag="loc")
                    if L == 0:
                        nc.sync.dma_start(loc[:], hown0[t * P:(t + 1) * P, 0:ncols])
                    elif t < SPLIT_T:
                        nc.sync.dma_start(loc[:], hlo[L][t * P:(t + 1) * P, 0:ncols])
                    else:
                        r0 = t * P - LO
                        nc.sync.dma_start(loc[:], hhi[L][r0:r0 + P, 0:ncols])

                    ndt = BF if L < 2 else F32
                    xs = sb.tile([P, ncols], ndt, tag="xs")
                    nc.vector.tensor_tensor(xs[:], po[:, 0:ncols],
                                            part_po[:, t, 0:ncols], ADD)
                    xn = sb.tile([P, ncols], ndt, tag="xn")
                    nc.vector.tensor_tensor(
                        xn[:].rearrange("p (h c) -> p h c", h=nH),
                        xs[:].rearrange("p (h c) -> p h c", h=nH),
                        rden[:].unsqueeze(2).to_broadcast([P, nH, ncols // nH]),
                        MUL)
                    t2 = sb.tile([P, ncols], ndt, tag="t2")
                    nc.vector.tensor_tensor(
                        t2[:].rearrange("p (h c) -> p h c", h=nH),
                        loc[:, 0:ncols].rearrange("p (h c) -> p h c", h=nH),
                        wr[:].unsqueeze(2).to_broadcast([P, nH, ncols // nH]),
                        MUL)
                    nc.vector.tensor_tensor(xn[:], xn[:], t2[:], ADD)
                    b_cur = (b0_sb, b1_sb, b2_sb)[L]
                    nc.vector.tensor_tensor(xn[:], xn[:], b_cur[:, 0:ncols], ADD)

                    if L < 2:
                        # E' = elu(xn)+1 = relu(xn) + exp(-relu(-xn))
                        m = sb.tile([P, ncols], BF, tag="m")
                        nc.scalar.activation(m[:], xn[:], RELU, scale=-1.0)
                        em = sb.tile([P, ncols], BF, tag="em")
                        nc.scalar.activation(em[:], m[:], EXP, scale=-1.0)
                        xr_ = sb.tile([P, ncols], BF, tag="xr_")
                        nc.scalar.activation(xr_[:], xn[:], RELU)
                        xe = sb.tile([P, ncols], BF, tag="xe")
                        nc.vector.tensor_tensor(xe[:], xr_[:], em[:], ADD)

                        # transpose to feature-major & next-layer h tile
                        xtt = sb.tile([P, 4, P], BF, tag="xtt")
                        for c4 in range(4):
                            pt = ps_pt.tile([P, P], BF, tag="stp")
                            nc.tensor.transpose(
                                pt[:], xe[:, c4 * P:(c4 + 1) * P], identb[:])
                            nc.scalar.copy(xtt[:, c4, :], pt[:])
                        phn = ps_h.tile([P, ROWG], F32, tag="ph")
                        n1 = min(512, rowg_n)
                        for kc in range(4):
                            nc.tensor.matmul(phn[:, 0:n1], lhsT=xtt[:, kc, :],
                                             rhs=W_n[:, kc, 0:n1],
                                             start=(kc == 0), stop=(kc == 3))
                        if rowg_n > 512:
                            for kc in range(4):
                                nc.tensor.matmul(phn[:, 512:rowg_n],
                                                 lhsT=xtt[:, kc, :],
                                                 rhs=W_n[:, kc, 512:rowg_n],
                                                 start=(kc == 0), stop=(kc == 3))
                        hn = sb.tile([P, rowg_n], BF, tag="hn")
                        nc.scalar.copy(hn[:, 0:min(528, rowg_n)], phn[:, 0:min(528, rowg_n)])
                        nxt_alow = 512 if L == 0 else 16
                        nh_n = 8 if L == 0 else 1
                        af_n = af1_sb if L == 0 else af2_sb
                        nc.vector.tensor_tensor(
                            a_nxt[:, t * 16:t * 16 + 2 * nh_n],
                            hn[:, nxt_alow:nxt_alow + 2 * nh_n],
                            af_n[:, 0:2 * nh_n], ADD)
                        if t < SPLIT_T:
                            nc.sync.dma_start(
                                hlo[L + 1][t * P:(t + 1) * P, :], hn[:])
                        else:
                            r0 = t * P - LO
                            nc.sync.dma_start(
                                hhi[L + 1][r0:r0 + P, :], hn[:])
                        if t == T - 1:
                            nc.gpsimd.collective_compute(
                                "AllGather", mybir.AluOpType.bypass,
                                replica_groups=rg, ins=[hhi[L + 1][:, :]],
                                outs=[thi[L + 1][:, :]])
                        if t == SPLIT_T - 1:
                            nc.gpsimd.collective_compute(
                                "AllGather", mybir.AluOpType.bypass,
                                replica_groups=rg, ins=[hlo[L + 1][:, :]],
                                outs=[tlo[L + 1][:, :]])
                    else:
                        rows = min(P, PER - t * P)
                        nc.sync.dma_start(out_d[t * P:t * P + rows, :],
                                          xn[:rows, 0:NCLS])

    nc.compile()
    nc.m = get_hw_module(nc.m)
    return nc


# ---------------------------------------------------------------------------
# Entry point
# ---------------------------------------------------------------------------

_CACHE = {}


def _get_nc(cfg):
    key = tuple(sorted((k, v) for k, v in cfg.items()))
    if key not in _CACHE:
        _CACHE[key] = build_gat_nc(cfg)
    return _CACHE[key]


def run(inputs, trace=False):
    cfg = real_cfg()
    in_maps = host_prepare(inputs, cfg)
    nc = _get_nc(cfg)
    res = bass_utils.run_bass_kernel_spmd(
        nc, in_maps, core_ids=list(range(cfg["R"])), trace=trace)
    out = np.concatenate([res.results[r]["out"] for r in range(cfg["R"])], axis=0)
    return out[: cfg["N"]], res


def kernel(**inputs) -> np.ndarray:
    out, _ = run(inputs, trace=False)
    return out.astype(np.float32)
